# revision 34
# baseline (speedup 1.0000x reference)
"""CPM3 attention kernel for 8 trn2 NeuronCores.

Sharding: hybrid batch+head parallel. Core c owns batch b=c//4 and the
4 heads (c%4)*4..+4 (two head-pair groups). Wo is row-sharded; the host
sums the 4 partial outputs per batch.

Key structure:
- host precomputes E = mask ? exp(position_bias) : 0 in fp16, so the
  device computes softmax numerators as p = exp(qk/8) * E. No PSUM bias
  inject, no mask DMA, no scalar_tensor_tensor.
- scores for a head pair land in one [128,1024] PSUM region -> a single
  wide EXP on the scalar engine, then a single 2x-mode DVE multiply.
- V is projected directly in [k, dh] layout (kv chunk as lhsT), so no
  transposes are needed; softmax denominators come from ones-columns
  interleaved in V (the PV matmul reduces over k, the partition axis).
- out projection accumulates both head-pair groups in PSUM; output DMA
  rides the gpsimd queue so semaphore waits never block the sync queue
  that streams E.
"""

import sys

sys.path.insert(0, "/opt/trn_rl_repo")

import numpy as np
import ml_dtypes

import concourse.bass as bass
import concourse.bacc as bacc
import concourse.tile as tile
import concourse.mybir as mybir
from concourse.bass_utils import run_bass_kernel_spmd

B, L, D, H, DH = 2, 2048, 1024, 16, 64
N_CORES = 8
HPC = 4  # heads per core
G = 2  # head-pair groups per core
QTS = 512
QN = L // QTS  # 4
KP = 128
KN = L // KP  # 16
DC = D // 128  # 8 contraction chunks
HVW = 2 * (DH + 1)  # 130 hv_aug cols per k-tile (2 heads x (64+ones))

F32 = mybir.dt.float32
F16 = mybir.dt.float16

_CACHE: dict = {}


def _build():
    if "nc" in _CACHE:
        return _CACHE["nc"]
    nc = bacc.Bacc("TRN2", target_bir_lowering=False, debug=False, num_devices=N_CORES)

    qT = nc.dram_tensor("qT", [DC, 128, L], F16, kind="ExternalInput").ap()
    kvT = nc.dram_tensor("kvT", [DC, 128, L], F16, kind="ExternalInput").ap()
    wq = nc.dram_tensor("wq", [G, 128, DC, 128], F16, kind="ExternalInput").ap()
    wk = nc.dram_tensor("wk", [G, 128, DC, 128], F16, kind="ExternalInput").ap()
    wv = nc.dram_tensor("wv", [G, 128, DC, 128], F16, kind="ExternalInput").ap()
    wo = nc.dram_tensor("wo", [128, G, D], F16, kind="ExternalInput").ap()
    Et = nc.dram_tensor("Et", [QN, G, 128, KN, 2 * QTS], F16, kind="ExternalInput").ap()
    indh = nc.dram_tensor("indh", [1, 256], F16, kind="ExternalInput").ap()
    out = nc.dram_tensor("out", [L, D], F16, kind="ExternalOutput").ap()

    with tile.TileContext(nc) as tc:
        with (
            tc.tile_pool(name="const", bufs=1) as constp,
            tc.tile_pool(name="eb", bufs=2) as ep,
            tc.tile_pool(name="stage", bufs=3) as stagep,
            tc.tile_pool(name="kcres", bufs=8) as kcp,
            tc.tile_pool(name="hq", bufs=2) as hqp,
            tc.tile_pool(name="hk", bufs=2) as hkp,
            tc.tile_pool(name="hv", bufs=2) as hvp,
            tc.tile_pool(name="sp", bufs=3) as spp,
            tc.tile_pool(name="pp", bufs=4) as ppp,
            tc.tile_pool(name="rc", bufs=4) as rcp,
            tc.tile_pool(name="bcs", bufs=2) as bcsp,
            tc.tile_pool(name="ctxn", bufs=4) as ctxnp,
            tc.tile_pool(name="outb", bufs=3) as outp,
            tc.tile_pool(name="psum", bufs=1, space=bass.MemorySpace.PSUM) as psp,
        ):
            def score_tile(name):
                return psp.tile([128, 2 * QTS], F32, tag="score", bufs=2, name=name)

            def ctx_tile(name):
                return psp.tile([128, QTS], F32, tag="ctx", bufs=4, name=name)

            # ---- first q chunk + wq first: unblock the first projections ----
            qcs = []
            qc0 = stagep.tile([128, L], F16, tag="stage", name="qc0")
            nc.sync.dma_start(qc0[:], qT[0])
            qcs.append(qc0)
            wq_t = constp.tile([128, G, DC, 128], F16, tag="wq")
            nc.sync.dma_start(wq_t[:, 0], wq[0])
            nc.sync.dma_start(wq_t[:, 1], wq[1])
            qc1 = stagep.tile([128, L], F16, tag="stage", name="qc1")
            nc.sync.dma_start(qc1[:], qT[1])
            qcs.append(qc1)
            wk_t = constp.tile([128, G, DC, 128], F16, tag="wk")
            nc.sync.dma_start(wk_t[:, 0], wk[0])
            nc.sync.dma_start(wk_t[:, 1], wk[1])
            wv_t = constp.tile([128, G, DC, 128], F16, tag="wv")
            nc.sync.dma_start(wv_t[:, 0], wv[0])
            nc.sync.dma_start(wv_t[:, 1], wv[1])
            wo_t = constp.tile([128, G, D], F16, tag="wo")
            nc.sync.dma_start(wo_t[:], wo[:])
            indh_t = constp.tile([1, 256], F16, tag="indh")
            nc.sync.dma_start(indh_t[:], indh[:])

            def fetch_e(qt, g, name):
                e_t = ep.tile([128, KN, 2 * QTS], F16, tag="E", name=name)
                nc.sync.dma_start(e_t[:], Et[qt, g])
                return e_t

            e_tiles = []

            # ---- q projections: one pass over qT, both groups at once ----
            # 8 qt-banks of accumulators: 3 score-tag tiles (6) + 2 ctx-tag
            def proj_accs(pfx):
                c4 = [ctx_tile(f"{pfx}c{i}") for i in range(QN)]
                s2 = [score_tile(f"{pfx}{i}") for i in range(2)]
                accs = {
                    0: [c4[0][:], c4[1][:], c4[2][:], c4[3][:]],
                    1: [s2[0][:, 0:QTS], s2[0][:, QTS : 2 * QTS],
                        s2[1][:, 0:QTS], s2[1][:, QTS : 2 * QTS]],
                }
                return accs

            hq_ps = proj_accs("hqp")
            for dc in range(DC):
                if dc < 2:
                    qc = qcs[dc]
                else:
                    qc = stagep.tile([128, L], F16, tag="stage", name=f"qc{dc}")
                    nc.sync.dma_start(qc[:], qT[dc])
                for g in range(G):
                    for qt in range(QN):
                        nc.tensor.matmul(
                            hq_ps[g][qt],
                            wq_t[:, g, dc, :],
                            qc[:, qt * QTS : (qt + 1) * QTS],
                            start=(dc == 0),
                            stop=(dc == DC - 1),
                        )
            hq_sb = {}
            for g in range(G):
                hq_sb[g] = hqp.tile([128, L], F16, tag="hq", name=f"hq_sb{g}")
                for qt in range(QN):
                    nc.scalar.copy(
                        hq_sb[g][:, qt * QTS : (qt + 1) * QTS],
                        hq_ps[g][qt],
                    )

            # ---- k projections: one pass over kvT; chunks stay resident ----
            kcs = []
            hk_ps = proj_accs("hkp")
            for dc in range(DC):
                kc = kcp.tile([128, L], F16, tag="kc", name=f"kc{dc}")
                nc.sync.dma_start(kc[:], kvT[dc])
                kcs.append(kc)
                for g in range(G):
                    for qt in range(QN):
                        nc.tensor.matmul(
                            hk_ps[g][qt],
                            wk_t[:, g, dc, :],
                            kc[:, qt * QTS : (qt + 1) * QTS],
                            start=(dc == 0),
                            stop=(dc == DC - 1),
                        )
            hk_sb = {}
            for g in range(G):
                hk_sb[g] = hkp.tile([128, L], F16, tag="hk", name=f"hk_sb{g}")
                for qt in range(QN):
                    nc.scalar.copy(
                        hk_sb[g][:, qt * QTS : (qt + 1) * QTS],
                        hk_ps[g][qt],
                    )

            # E streams start only after the prologue input DMAs are queued
            e_tiles.append(fetch_e(0, 0, "e_p0"))
            e_tiles.append(fetch_e(0, 1, "e_p1"))

            # ---- v projections, directly in [k, dh] layout ----
            # out[k, dh] = sum_d kv[d, k] * Wv[dh, d]: kv chunk is lhsT.
            hv_sb = {}
            for g in range(G):
                hv_sb[g] = hvp.tile(
                    [128, KN * HVW + 64], F16, tag="hv", name=f"hv_sb{g}"
                )
                nc.gpsimd.memset(hv_sb[g][:].bitcast(mybir.dt.uint16), 0x3C00)
            for g in range(G):
                for kt in range(KN):
                    vt = ctx_tile(f"vt{g}_{kt}")
                    for dc in range(DC):
                        nc.tensor.matmul(
                            vt[:, 0:128],
                            kcs[dc][:, kt * KP : (kt + 1) * KP],
                            wv_t[:, g, dc, :],
                            start=(dc == 0),
                            stop=(dc == DC - 1),
                        )
                    o = kt * HVW
                    nc.scalar.copy(hv_sb[g][:, o : o + DH], vt[:, 0:DH])
                    nc.vector.tensor_copy(
                        hv_sb[g][:, o + DH + 1 : o + 2 * DH + 1], vt[:, DH:128]
                    )

            # ---- main loop: passes over (qt, head-pair group) ----
            def emit_group_epilogue(g, qt, ctx, rcr):
                bc = score_tile(f"bc{g}_{qt}")
                for hh in range(2):
                    nc.tensor.matmul(
                        bc[:, 0:QTS],
                        indh_t[:, hh * 128 : (hh + 1) * 128],
                        rcr[hh][:],
                        start=(hh == 0),
                        stop=(hh == 1),
                    )
                bc_sb = bcsp.tile([128, QTS], F32, tag="bcs", name=f"bc_sb{g}_{qt}")
                nc.vector.tensor_copy(bc_sb[:], bc[:, 0:QTS])
                ctxn = ctxnp.tile([128, QTS], F16, tag="ctxn", name=f"ctxn{g}_{qt}")
                for hh in range(2):
                    nc.vector.tensor_tensor(
                        ctxn[hh * DH : (hh + 1) * DH, :],
                        ctx[hh][0:DH, :],
                        bc_sb[hh * DH : (hh + 1) * DH, :],
                        mybir.AluOpType.mult,
                    )
                return ctxn

            def emit_outproj_qs(qt, ctxn_pair, qs):
                op = score_tile(f"op{qt}_{qs}")
                for oh in range(2):
                    for g in range(G):
                        nc.tensor.matmul(
                            op[:, oh * QTS : (oh + 1) * QTS],
                            ctxn_pair[g][:, qs * 128 : (qs + 1) * 128],
                            wo_t[:, g, oh * QTS : (oh + 1) * QTS],
                            start=(g == 0),
                            stop=(g == G - 1),
                        )
                ob = outp.tile([128, D], F16, tag="outb", name=f"ob{qt}_{qs}")
                nc.vector.tensor_copy(ob[:, 0:QTS], op[:, 0:QTS])
                nc.scalar.copy(ob[:, QTS : 2 * QTS], op[:, QTS : 2 * QTS])
                r0 = qt * QTS + qs * 128
                eng = nc.gpsimd if qs % 2 == 0 else nc.sync
                eng.dma_start(out[r0 : r0 + 128, :], ob[:])

            def emit_outproj(qt, ctxn_pair):
                for qs in range(QN):
                    emit_outproj_qs(qt, ctxn_pair, qs)

            def emit_recip(g, qt, ctx):
                rcr = []
                for hh in range(2):
                    dsb = rcp.tile([1, QTS], F32, tag="dsb", name=f"dsb{g}_{qt}_{hh}")
                    nc.vector.tensor_copy(dsb[:], ctx[hh][DH : DH + 1, :])
                    rcf = rcp.tile([1, QTS], F32, tag="rcf", name=f"rcf{g}_{qt}_{hh}")
                    nc.vector.reciprocal_approx_fast(rcf[:], dsb[:])
                    rc16 = rcp.tile([1, QTS], F16, tag="rcr", name=f"rcr{g}_{qt}_{hh}")
                    nc.vector.tensor_copy(rc16[:], rcf[:])
                    rcr.append(rc16)
                return rcr

            passes = [(qt, g) for qt in range(QN) for g in range(G)]
            pending_pv = []
            pending_recip = None  # (g, qt, ctx) awaiting its DVE recip chain
            pending_group = None  # (g, qt, ctx, rcr) awaiting bc/ctxn
            ctxn_done = {}
            pending_out = None

            for pi, (qt, g) in enumerate(passes):
                e_t = e_tiles[pi]
                if pi + 2 < len(passes):
                    nqt, ng = passes[pi + 2]
                    e_tiles.append(fetch_e(nqt, ng, f"e_p{pi + 2}"))

                ctx = [ctx_tile(f"ctx{g}_{qt}_{hh}") for hh in range(2)]
                for kt in range(KN):
                    score = score_tile(f"sc{g}_{qt}_{kt}")
                    for hh in range(2):
                        nc.tensor.matmul(
                            score[:, hh * QTS : (hh + 1) * QTS],
                            hk_sb[g][
                                hh * DH : (hh + 1) * DH, kt * KP : (kt + 1) * KP
                            ],
                            hq_sb[g][
                                hh * DH : (hh + 1) * DH, qt * QTS : (qt + 1) * QTS
                            ],
                            start=True,
                            stop=True,
                        )
                    while len(pending_pv) >= 2:
                        pg, pkt, pctx, pp_t = pending_pv.pop(0)
                        for hh in range(2):
                            nc.tensor.matmul(
                                pctx[hh][:],
                                hv_sb[pg][
                                    :, pkt * HVW + hh * 65 : pkt * HVW + hh * 65 + 128
                                ],
                                pp_t[:, hh * QTS : (hh + 1) * QTS],
                                start=(pkt == 0),
                                stop=(pkt == KN - 1),
                            )
                    if kt == 4 and pending_group is not None:
                        pg_, pqt_, pctx_, prcr_ = pending_group
                        ctxn_done.setdefault(pqt_, []).append(
                            emit_group_epilogue(pg_, pqt_, pctx_, prcr_)
                        )
                        pending_group = None
                    if kt in (6, 8, 10, 12) and pending_out is not None:
                        qs = (kt - 6) // 2
                        emit_outproj_qs(pending_out, ctxn_done[pending_out], qs)
                        if qs == QN - 1:
                            ctxn_done.pop(pending_out)
                            pending_out = None

                    s_t = spp.tile([128, 2 * QTS], F16, tag="s", name=f"s{g}_{qt}_{kt}")
                    nc.scalar.activation(
                        s_t[:], score[:], mybir.ActivationFunctionType.Exp
                    )
                    p_t = ppp.tile([128, 2 * QTS], F16, tag="p", name=f"p{g}_{qt}_{kt}")
                    nc.vector.tensor_tensor(
                        p_t[:], s_t[:], e_t[:, kt, :], mybir.AluOpType.mult
                    )
                    pending_pv.append((g, kt, ctx, p_t))
                    # previous pass's recip chain goes on the DVE queue only
                    # after this pass's pipeline is rolling (post mult kt2)
                    if kt == 2 and pending_recip is not None:
                        pg_, pqt_, pctx_ = pending_recip
                        pending_group = (pg_, pqt_, pctx_, emit_recip(pg_, pqt_, pctx_))
                        pending_recip = None

                if pi == len(passes) - 1:
                    for pg, pkt, pctx, pp_t in pending_pv:
                        for hh in range(2):
                            nc.tensor.matmul(
                                pctx[hh][:],
                                hv_sb[pg][
                                    :, pkt * HVW + hh * 65 : pkt * HVW + hh * 65 + 128
                                ],
                                pp_t[:, hh * QTS : (hh + 1) * QTS],
                                start=(pkt == 0),
                                stop=(pkt == KN - 1),
                            )
                    pending_pv = []

                pending_recip = (g, qt, ctx)
                if g == G - 1:
                    pending_out = qt

            # tail: last pass's epilogue + final out projection
            pg_, pqt_, pctx_ = pending_recip
            rcr_ = emit_recip(pg_, pqt_, pctx_)
            if pending_group is not None:
                g2, q2, c2, r2 = pending_group
                ctxn_done.setdefault(q2, []).append(emit_group_epilogue(g2, q2, c2, r2))
            ctxn_done.setdefault(pqt_, []).append(
                emit_group_epilogue(pg_, pqt_, pctx_, rcr_)
            )
            emit_outproj(pending_out, ctxn_done.pop(pending_out))

    nc.compile()
    _CACHE["nc"] = nc
    return nc


def _prep_shared(query, key_value, mask, position_bias):
    shared = {"qT": {}, "kvT": {}}
    for b in range(B):
        shared["qT"][b] = np.ascontiguousarray(
            query[b].reshape(L, DC, 128).transpose(1, 2, 0)
        ).astype(np.float16)
        shared["kvT"][b] = np.ascontiguousarray(
            key_value[b].reshape(L, DC, 128).transpose(1, 2, 0)
        ).astype(np.float16)
    expb = np.exp(position_bias, dtype=np.float32)  # [H, L, L]
    m = np.asarray(mask, dtype=bool)
    shared["E16"] = {b: (expb * m[b][None]).astype(np.float16) for b in range(B)}
    indh = np.concatenate(
        [
            np.where(np.arange(128) < 64, 1.0, 0.0),
            np.where(np.arange(128) >= 64, 1.0, 0.0),
        ]
    ).astype(np.float16)[None, :]
    shared["indh"] = np.ascontiguousarray(indh)
    return shared


def _prep_core(core, Wq, Wk, Wv, Wo, shared):
    b = core // 4
    h0 = (core % 4) * HPC

    def packw(w, scale=1.0):
        rows = w[h0 * DH : (h0 + HPC) * DH]  # [256, D]
        return np.ascontiguousarray(
            (rows.T * scale).reshape(DC, 128, 2, 128).transpose(2, 1, 0, 3)
        ).astype(np.float16)

    Ec = shared["E16"][b][h0 : h0 + HPC]  # [4, L(q), L(k)] fp16
    # [g2, hh2, qt4, qf512, kt16, kp128] -> [qt, g, kp, kt, hh, qf]
    Ep = np.ascontiguousarray(
        Ec.reshape(G, 2, QN, QTS, KN, KP).transpose(2, 0, 5, 4, 1, 3)
    ).reshape(QN, G, 128, KN, 2 * QTS)
    wo_rows = Wo[:, h0 * DH : (h0 + HPC) * DH]  # [D, 256]
    wo_p = np.ascontiguousarray(
        wo_rows.T.reshape(G, 128, D).transpose(1, 0, 2)
    ).astype(np.float16)
    return {
        "qT": shared["qT"][b],
        "kvT": shared["kvT"][b],
        "wq": packw(Wq, 1.0 / np.sqrt(DH)),
        "wk": packw(Wk),
        "wv": packw(Wv),
        "wo": wo_p,
        "Et": Ep,
        "indh": shared["indh"],
    }


def kernel(query, key_value, mask, position_bias, Wq, Wk, Wv, Wo, _trace=False):
    query = np.asarray(query, dtype=np.float32)
    key_value = np.asarray(key_value, dtype=np.float32)
    mask = np.asarray(mask)
    position_bias = np.asarray(position_bias, dtype=np.float32)
    Wq = np.asarray(Wq, dtype=np.float32)
    Wk = np.asarray(Wk, dtype=np.float32)
    Wv = np.asarray(Wv, dtype=np.float32)
    Wo = np.asarray(Wo, dtype=np.float32)

    nc = _build()
    shared = _prep_shared(query, key_value, mask, position_bias)
    in_maps = [_prep_core(c, Wq, Wk, Wv, Wo, shared) for c in range(N_CORES)]
    res = run_bass_kernel_spmd(nc, in_maps, list(range(N_CORES)), trace=_trace)
    _CACHE["last_result"] = res
    full = np.zeros((B, L, D), dtype=np.float64)
    for c in range(N_CORES):
        full[c // 4] += res.results[c]["out"]
    return full.astype(np.float32)
